# revision 6
# baseline (speedup 1.0000x reference)
import os

import numpy as np

import concourse.bass as bass
from concourse import bacc
import concourse.mybir as mybir
import concourse.tile as tile
from concourse.bass_utils import run_bass_kernel_spmd
from concourse.masks import make_identity

AF = mybir.ActivationFunctionType
ALU = mybir.AluOpType
F32 = mybir.dt.float32

B, N, DIM = 4, 8192, 256
H, HD, INNER = 8, 64, 512
WS = 128
J = 256
SCALE = DIM ** -0.5
NCORES = 8
NQ = N // 2
NWC = NQ // WS
NKV = NQ + WS
NBLK = NKV // WS
BIG = 1.0e30
WG = 4
NG = NWC // WG


def _build_program(trace_scopes=False):
    nc = bacc.Bacc("TRN2", target_bir_lowering=False, debug=False)

    xT = nc.dram_tensor("xT", [2, 128, NKV], F32, kind="ExternalInput")
    wqT = nc.dram_tensor("wqT", [2, 128, INNER], F32, kind="ExternalInput")
    wkT = nc.dram_tensor("wkT", [2, 128, INNER], F32, kind="ExternalInput")
    wvT = nc.dram_tensor("wvT", [2, 128, INNER], F32, kind="ExternalInput")
    wfcT = nc.dram_tensor("wfcT", [4, 128, DIM], F32, kind="ExternalInput")
    kcol = nc.dram_tensor("kcol", [4, NKV], F32, kind="ExternalInput")
    qm = nc.dram_tensor("qm", [128, 4, NWC], F32, kind="ExternalInput")
    qmg = nc.dram_tensor("qmg", [128, 4, NWC], F32, kind="ExternalInput")
    qmu = nc.dram_tensor("qmu", [128, 4, NWC], F32, kind="ExternalInput")
    sqc = nc.dram_tensor("sqc", [128, 128], F32, kind="ExternalInput")
    skc = nc.dram_tensor("skc", [128, 128], F32, kind="ExternalInput")
    vfm = nc.dram_tensor("vfm", [128, 1], F32, kind="ExternalInput")
    vfa = nc.dram_tensor("vfa", [128, 1], F32, kind="ExternalInput")

    attn_o = nc.dram_tensor("attn_o", [H, NWC, WS, J], F32, kind="ExternalOutput")
    y_o = nc.dram_tensor("y_o", [NWC, WS, DIM], F32, kind="ExternalOutput")

    with tile.TileContext(nc) as tc:
        with (
            tc.tile_pool(name="const", bufs=1) as cpool,
            tc.tile_pool(name="vall", bufs=1) as vpool,
            tc.tile_pool(name="xpool", bufs=1) as xpool,
            tc.tile_pool(name="proj", bufs=2) as ppool,
            tc.tile_pool(name="work", bufs=3) as wpool,
            tc.tile_pool(name="psum", bufs=2, space="PSUM") as pspool,
            tc.tile_pool(name="psum_y", bufs=1, space="PSUM") as ypool,
        ):
            xT_sb = xpool.tile([128, 2, NKV], F32, tag="xT")
            nc.sync.dma_start(out=xT_sb, in_=xT.rearrange("c p n -> p c n"))

            wq_sb = cpool.tile([128, 2, INNER], F32, tag="wq")
            wk_sb = cpool.tile([128, 2, INNER], F32, tag="wk")
            wv_sb = cpool.tile([128, 2, INNER], F32, tag="wv")
            nc.sync.dma_start(out=wq_sb, in_=wqT.rearrange("c p n -> p c n"))
            nc.sync.dma_start(out=wk_sb, in_=wkT.rearrange("c p n -> p c n"))
            nc.sync.dma_start(out=wv_sb, in_=wvT.rearrange("c p n -> p c n"))
            wfc_sb = cpool.tile([128, 4, DIM], F32, tag="wfc")
            nc.sync.dma_start(out=wfc_sb, in_=wfcT.rearrange("c p n -> p c n"))

            kcol_sb = [cpool.tile([1, NKV], F32, tag=f"kcol{r}", name=f"kcol{r}") for r in range(4)]
            for r in range(4):
                nc.sync.dma_start(out=kcol_sb[r], in_=kcol[r : r + 1, :])

            qm_sb = cpool.tile([128, 4, NWC], F32, tag="qm")
            qmg_sb = cpool.tile([128, 4, NWC], F32, tag="qmg")
            qmu_sb = cpool.tile([128, 4, NWC], F32, tag="qmu")
            nc.sync.dma_start(out=qm_sb, in_=qm[:, :, :])
            nc.sync.dma_start(out=qmg_sb, in_=qmg[:, :, :])
            nc.sync.dma_start(out=qmu_sb, in_=qmu[:, :, :])

            sq_sb = cpool.tile([128, 128], F32, tag="sq")
            sk_sb = cpool.tile([128, 128], F32, tag="sk")
            nc.sync.dma_start(out=sq_sb, in_=sqc[:, :])
            nc.sync.dma_start(out=sk_sb, in_=skc[:, :])

            vf_m = cpool.tile([128, 1], F32, tag="vfm")
            vf_a = cpool.tile([128, 1], F32, tag="vfa")
            nc.sync.dma_start(out=vf_m, in_=vfm[:, :])
            nc.sync.dma_start(out=vf_a, in_=vfa[:, :])

            ones_sb = cpool.tile([1, 128], F32, tag="ones")
            nc.gpsimd.memset(ones_sb, 1.0)
            ident = cpool.tile([128, 128], F32, tag="ident")
            make_identity(nc, ident)

            v_all = vpool.tile([128, NBLK, H, HD], F32, tag="vall")
            for t in range(NBLK):
                pv = pspool.tile([128, INNER], F32, tag="eb")
                nc.tensor.matmul(
                    pv, lhsT=xT_sb[:, 0, t * 128 : (t + 1) * 128], rhs=wv_sb[:, 0, :],
                    start=True, stop=False)
                nc.tensor.matmul(
                    pv, lhsT=xT_sb[:, 1, t * 128 : (t + 1) * 128], rhs=wv_sb[:, 1, :],
                    start=False, stop=True)
                nc.vector.tensor_copy(
                    v_all[:, t, :, :].rearrange("p h d -> p (h d)"),
                    pv.rearrange("p (pr q d) -> p (pr q d)", pr=4, q=2, d=HD),
                )
            blk0 = v_all[:, 0, :, :].rearrange("p h d -> p (h d)")
            nc.vector.tensor_scalar(
                out=blk0, in0=blk0, scalar1=vf_m[:, 0:1], scalar2=vf_a[:, 0:1],
                op0=ALU.mult, op1=ALU.add)

            for g in range(NG):
                q_lo = (g * WG) * WS
                kv_lo = q_lo
                nkg = WG * WS + WS

                y_ps = [ypool.tile([128, DIM], F32, tag=f"y{i}", name=f"yps{i}") for i in range(WG)]

                for pair in range(4):
                    qTg = ppool.tile([128, WG * WS], F32, tag="qTg")
                    kTg = ppool.tile([128, nkg], F32, tag="kTg")
                    for t in range(WG * WS // 512):
                        pq = pspool.tile([128, 512], F32, tag="eb")
                        for c in range(2):
                            nc.tensor.matmul(
                                pq,
                                lhsT=wq_sb[:, c, pair * 128 : (pair + 1) * 128],
                                rhs=xT_sb[:, c, WS + q_lo + t * 512 : WS + q_lo + (t + 1) * 512],
                                start=(c == 0), stop=(c == 1))
                        nc.vector.tensor_copy(qTg[:, t * 512 : (t + 1) * 512], pq)
                    for t in range(2):
                        w0 = t * 512
                        wid = min(512, nkg - w0)
                        pk = pspool.tile([128, 512], F32, tag="eb")
                        for c in range(2):
                            nc.tensor.matmul(
                                pk[:, :wid],
                                lhsT=wk_sb[:, c, pair * 128 : (pair + 1) * 128],
                                rhs=xT_sb[:, c, kv_lo + w0 : kv_lo + w0 + wid],
                                start=(c == 0), stop=(c == 1))
                        nc.vector.tensor_copy(kTg[:, w0 : w0 + wid], pk[:, :wid])

                    outT_sb = [None] * WG
                    for wl in range(WG):
                        w = g * WG + wl
                        outT_sb[wl] = wpool.tile([128, WS], F32, tag="outT", name="outT")
                        for hq in range(2):
                            h = pair + 4 * hq
                            hp = 64 * hq
                            e_ps = pspool.tile([128, J], F32, tag="eb")
                            nc.tensor.matmul(
                                e_ps,
                                lhsT=qTg[hp : hp + 64, wl * WS : (wl + 1) * WS],
                                rhs=kTg[hp : hp + 64, wl * WS : wl * WS + J],
                                start=True, stop=False)
                            nc.tensor.matmul(
                                e_ps,
                                lhsT=ones_sb,
                                rhs=kcol_sb[pair][0:1, kv_lo + wl * WS : kv_lo + wl * WS + J],
                                start=False, stop=False, skip_group_check=True)
                            nc.tensor.matmul(
                                e_ps[:, WS:J],
                                lhsT=sq_sb, rhs=sk_sb,
                                start=False, stop=True, skip_group_check=True)

                            p2 = wpool.tile([128, J], F32, tag="p2")
                            rs = wpool.tile([128, 4], F32, tag="rs")
                            nc.scalar.activation(
                                out=p2, in_=e_ps, func=AF.Exp, scale=SCALE,
                                accum_out=rs[:, 0:1])
                            nc.vector.tensor_scalar(
                                out=rs[:, 1:2], in0=rs[:, 0:1],
                                scalar1=qmg_sb[:, pair, w : w + 1], scalar2=None,
                                op0=ALU.add)
                            nc.vector.reciprocal(rs[:, 2:3], rs[:, 1:2])
                            nc.vector.tensor_scalar(
                                out=rs[:, 3:4], in0=rs[:, 2:3],
                                scalar1=qm_sb[:, pair, w : w + 1], scalar2=None,
                                op0=ALU.mult)
                            attn_sb = wpool.tile([128, J], F32, tag="attn")
                            nc.vector.tensor_scalar(
                                out=attn_sb, in0=p2,
                                scalar1=rs[:, 3:4],
                                scalar2=qmu_sb[:, pair, w : w + 1],
                                op0=ALU.mult, op1=ALU.add)
                            nc.sync.dma_start(out=attn_o[h, w], in_=attn_sb)

                            at_ps = pspool.tile([128, J], F32, tag="tb")
                            nc.tensor.transpose(at_ps[:, 0:WS], attn_sb[:, 0:WS], ident)
                            nc.tensor.transpose(at_ps[:, WS:J], attn_sb[:, WS:J], ident)
                            atT = wpool.tile([128, J], F32, tag="attnT")
                            nc.vector.tensor_copy(atT, at_ps)

                            o_ps = pspool.tile([64, WS], F32, tag="tb")
                            slot = pair * 2 + hq
                            nc.tensor.matmul(
                                o_ps,
                                lhsT=v_all[:, g * WG + wl, slot, :],
                                rhs=atT[:, 0:WS], start=True, stop=False)
                            nc.tensor.matmul(
                                o_ps,
                                lhsT=v_all[:, g * WG + wl + 1, slot, :],
                                rhs=atT[:, WS:J], start=False, stop=True)
                            nc.scalar.copy(out=outT_sb[wl][hp : hp + 64, :], in_=o_ps)

                        nc.tensor.matmul(
                            y_ps[wl],
                            lhsT=outT_sb[wl],
                            rhs=wfc_sb[:, pair, :],
                            start=(pair == 0), stop=(pair == 3))

                for wl in range(WG):
                    y_sb = wpool.tile([128, DIM], F32, tag="ysb")
                    nc.vector.tensor_copy(y_sb, y_ps[wl])
                    nc.sync.dma_start(out=y_o[g * WG + wl], in_=y_sb)

    nc.compile()
    return nc


_PROGRAM = None


def _get_program():
    global _PROGRAM
    if _PROGRAM is None:
        _PROGRAM = _build_program()
    return _PROGRAM


def _host_prep(x, mask, wq, wk, wv, w_fc):
    xf = np.asarray(x, dtype=np.float32)
    maskf = np.asarray(mask).astype(np.float32)

    perm = []
    for pair in range(4):
        for hq in range(2):
            h = pair + 4 * hq
            perm.extend(range(h * HD, (h + 1) * HD))
    perm = np.array(perm)

    wqT = np.ascontiguousarray(wq[perm, :].T).reshape(2, 128, INNER)
    wkT = np.ascontiguousarray(wk[perm, :].T).reshape(2, 128, INNER)
    wvT = np.ascontiguousarray(wv[perm, :].T).reshape(2, 128, INNER)
    wfcT = np.empty((4, 128, DIM), dtype=np.float32)
    for pair in range(4):
        wfcT[pair, 0:64] = w_fc[:, (pair) * HD : (pair + 1) * HD].T
        wfcT[pair, 64:128] = w_fc[:, (pair + 4) * HD : (pair + 5) * HD].T

    rr, ii = np.meshgrid(np.arange(128), np.arange(128), indexing="ij")
    sq = (ii <= rr).astype(np.float32)
    sk = np.where(ii > rr, -np.float32(BIG), 0.0).astype(np.float32)

    in_maps = []
    for c in range(NCORES):
        bi, s = divmod(c, 2)
        q0 = s * NQ
        xkv = np.zeros((NKV, DIM), dtype=np.float32)
        lo = q0 - WS
        src_lo = max(lo, 0)
        xkv[src_lo - lo :] = xf[bi, src_lo : q0 + NQ]
        xT = np.ascontiguousarray(xkv.T).reshape(2, 128, NKV)

        mkv = np.zeros((4, NKV), dtype=np.float32)
        mkv[:, src_lo - lo :] = maskf[:, src_lo : q0 + NQ]
        kcol = (mkv - 1.0) * np.float32(BIG)

        mq = maskf[:, q0 : q0 + NQ].reshape(4, NWC, WS)
        qm_h = np.ascontiguousarray(mq.transpose(2, 0, 1))
        qmg_h = 1.0 - qm_h
        qmu_h = qmg_h / 256.0

        vf_mul = np.full((128, 1), 0.0 if s == 0 else 1.0, dtype=np.float32)
        vf_add = np.full((128, 1), -1.0 if s == 0 else 0.0, dtype=np.float32)

        in_maps.append({
            "xT": xT, "wqT": wqT, "wkT": wkT, "wvT": wvT, "wfcT": wfcT,
            "kcol": np.ascontiguousarray(kcol),
            "qm": qm_h, "qmg": np.ascontiguousarray(qmg_h),
            "qmu": np.ascontiguousarray(qmu_h),
            "sqc": sq, "skc": sk, "vfm": vf_mul, "vfa": vf_add,
        })
    return in_maps


def kernel(x, mask, wq, wk, wv, w_fc, b_fc):
    nc = _get_program()
    in_maps = _host_prep(x, mask, wq, wk, wv, w_fc)

    trace = bool(int(os.environ.get("KERNEL_TRACE", "0")))
    res = run_bass_kernel_spmd(nc, in_maps, core_ids=list(range(NCORES)), trace=trace)
    if trace and res.exec_time_ns is not None:
        kernel.last_exec_time_ns = res.exec_time_ns
        kernel.last_results = res

    out = np.empty((B, N, DIM), dtype=np.float32)
    attn = np.empty((B, H, N // WS, WS, J), dtype=np.float32)
    bfc = np.asarray(b_fc, dtype=np.float32)
    for c in range(NCORES):
        bi, s = divmod(c, 2)
        r = res.results[c]
        out[bi, s * NQ : (s + 1) * NQ] = r["y_o"].reshape(NQ, DIM) + bfc
        attn[bi, :, s * NWC : (s + 1) * NWC] = r["attn_o"]
    return out, attn
